# revision 11
# baseline (speedup 1.0000x reference)
"""Trainium2 Bass kernel for nn_ExchangeBlock (GNN message passing / e3nn-style
tensor-product edge block), SPMD across 8 NeuronCores.

Sharding: edges across the 8 cores; node features and params replicated.

Per core the kernel runs in two phases so the ScalarEngine activation table is
switched exactly once (table loads cost ~2.7us):
  Phase A (exp table):   geometry + Gaussian RBF for every 512-edge block.
    sqrt(d2) is computed on the VectorEngine via Newton rsqrt (bit-trick
    seed), RBF values are parked in a resident SBUF buffer.
  Phase B (silu table):  gathers of node features, outer-product tensor
    product features P[e,1344] (DVE broadcast ops), PE transposes + matmul
    against pre-flattened weights, LayerNorm (stats on DVE, rstd via Newton,
    affine folded into a widened dfilter matmul), dfilter MLP, final MLP.
    The cosine cutoff envelope uses Sin (resident in the silu table set).

Biases enter PSUM via rank-1 (K=1) matmuls; the final [512]->1 projection is
a fused DVE multiply+reduce.
"""

import sys

sys.path.insert(0, "/opt/trn_rl_repo")

import numpy as np

import concourse.bass as bass
import concourse.mybir as mybir
import concourse.tile as tile
from concourse import bacc
from concourse.bass import IndirectOffsetOnAxis
from concourse.bass_utils import run_bass_kernel_spmd
from concourse.masks import make_identity

F32 = mybir.dt.float32
BF16 = mybir.dt.bfloat16
I32 = mybir.dt.int32
AF = mybir.ActivationFunctionType
OP = mybir.AluOpType

# Problem constants
L0, L1, L2 = 32, 16, 8
NS = 128
NB = 64
CUTOFF = 7.0
N_NODES = 50000
N_EDGES = 400000
NODE_DIM = 120
NCORES = 8

NEXT = 124            # packed node row: 120 feats + 3 pos + 1 graph_batch
BLK = 512             # edges per block
SUB = 4               # 128-edge sub-tiles per block
P = 128
KTP = 1344            # 1024 + 256 + 64 contraction size
KPAD = 1408           # padded to 11 chunks of 128
NCHUNK = 11
RSQRT_MAGIC = 0x5F3759DF

E_CORE = N_EDGES // NCORES                      # 50000
NBLOCKS = (E_CORE + BLK - 1) // BLK             # 98
E_PAD = NBLOCKS * BLK                           # 50176

_compiled = None


def _patch_walrus_dge_levels():
    """This walrus build compiles with DynamicDMA disabled by default, which
    makes indirect (vector-dynamic-offset) DMAs crash the exec unit. Append
    the full --dge-levels set to every walrus invocation."""
    import concourse.bass_utils as _bu

    if getattr(_bu, "_dge_patched", False):
        return
    orig = _bu.run_command

    def patched(argv, **kw):
        if argv and "walrus_driver" in str(argv[0]) and not any(
            "dge-levels" in str(a) for a in argv
        ):
            argv = list(argv) + [
                "--dge-levels=io,spill_reload,scalar_dynamic_offset,"
                "vector_dynamic_offsets,dynamic_size,dst_reduce,transpose"
            ]
        return orig(argv, **kw)

    _bu.run_command = patched
    _bu._dge_patched = True


_patch_walrus_dge_levels()


def _patch_drain_and_barrier():
    """The final Tile drain runs on the SP engine, whose Drain lowering in this
    walrus build has no free sync-wait slots (its HWDGE queue waits fill them).
    Hoist the tile-clock waits onto dedicated nop instructions emitted just
    before the drain, one wait per nop."""
    if getattr(tile.TileContext, "_dab_patched", False):
        return
    orig = tile.TileContext._drain_and_barrier

    def patched(self, tick_clock, wait_clock):
        nc = self.nc
        nops = [nc.sync.nop() for _ in range(32)]
        orig_ret = None
        drain_inst = nc.sync.drain()
        from concourse.tile import ScopedClock

        wait_clock.add_sem_waits(
            drain_inst.ins, ScopedClock({None: tick_clock.global_clock})
        )
        si = drain_inst.ins.sync_info
        waits = list(si.on_wait) if si and si.on_wait else []
        if waits:
            assert len(waits) <= len(nops), f"{len(waits)} waits > nop slots"
            si.on_wait = []
            for w, n in zip(waits, nops):
                n.ins.sync_info = mybir.SyncInfo(on_wait=[w], on_update=[])

        nc.all_engine_barrier()
        assert self.sems is not None
        popped = nc._tile_sem_poison_stack.pop()
        assert popped is self._sem_poison
        nc.clear_and_free_semaphores(list(self.sems.allocated().values()))
        nc.all_engine_barrier()
        return orig_ret

    tile.TileContext._drain_and_barrier = patched
    tile.TileContext._dab_patched = True


_patch_drain_and_barrier()


def _newton_rsqrt(nc, pool, u, n, magic_t, one_i, tag):
    """rsqrt(u) for u[:, :n] > 0 on the VectorEngine (no ScalarE table)."""
    bits = pool.tile([P, n], I32, tag=f"{tag}_b")
    nc.vector.tensor_copy(out=bits[:].bitcast(F32), in_=u)  # reinterpret: copy raw
    # t = bits >> 1 ; y0bits = magic - t
    nc.vector.tensor_scalar(
        out=bits[:], in0=bits[:], scalar1=1, scalar2=None,
        op0=OP.arith_shift_right,
    )
    yb = pool.tile([P, n], I32, tag=f"{tag}_y")
    nc.vector.tensor_tensor(
        out=yb[:], in0=magic_t[:, 0:1].to_broadcast([P, n]), in1=bits[:],
        op=OP.subtract,
    )
    y = yb[:].bitcast(F32)
    t1 = pool.tile([P, n], F32, tag=f"{tag}_t1")
    for _ in range(3):
        nc.vector.tensor_mul(t1[:], y, y)
        nc.vector.tensor_mul(t1[:], t1[:], u)
        nc.vector.tensor_scalar(
            out=t1[:], in0=t1[:], scalar1=-0.5, scalar2=1.5, op0=OP.mult, op1=OP.add,
        )
        nc.vector.tensor_mul(y, y, t1[:])
    return yb


def _build(nblocks: int):
    nc = bacc.Bacc("TRN2", target_bir_lowering=False, debug=False)

    nodes_ext = nc.dram_tensor("nodes_ext", (N_NODES, NEXT), F32, kind="ExternalInput").ap()
    posgb = nc.dram_tensor("posgb", (N_NODES, 4), F32, kind="ExternalInput").ap()
    cell9 = nc.dram_tensor("cell9", (32, 9), F32, kind="ExternalInput").ap()
    srcidx = nc.dram_tensor("srcidx", (nblocks * BLK,), I32, kind="ExternalInput").ap()
    dstidx = nc.dram_tensor("dstidx", (nblocks * BLK,), I32, kind="ExternalInput").ap()
    eshift = nc.dram_tensor("eshift", (nblocks * BLK, 3), F32, kind="ExternalInput").ap()
    wflat = nc.dram_tensor("wflat", (KPAD, NS), F32, kind="ExternalInput").ap()
    dfw1 = nc.dram_tensor("dfw1", (NB, 128), F32, kind="ExternalInput").ap()
    dfb1 = nc.dram_tensor("dfb1", (1, 128), F32, kind="ExternalInput").ap()
    dfw2gb = nc.dram_tensor("dfw2gb", (128, 256), F32, kind="ExternalInput").ap()
    dfb2gb = nc.dram_tensor("dfb2gb", (1, 256), F32, kind="ExternalInput").ap()
    mlpw1 = nc.dram_tensor("mlpw1", (128, 512), F32, kind="ExternalInput").ap()
    mlpb1 = nc.dram_tensor("mlpb1", (1, 512), F32, kind="ExternalInput").ap()
    w2row = nc.dram_tensor("w2row", (1, 512), F32, kind="ExternalInput").ap()
    b2sc = nc.dram_tensor("b2sc", (1, 1), F32, kind="ExternalInput").ap()
    offs = nc.dram_tensor("offs", (1, NB), F32, kind="ExternalInput").ap()
    out = nc.dram_tensor("out", (nblocks * BLK,), F32, kind="ExternalOutput").ap()

    width = CUTOFF / (NB - 1)
    coeff = 0.5 / (width * width)       # rbf = exp(-coeff*(d-off)^2)
    sqc = float(np.sqrt(coeff))

    with tile.TileContext(nc) as tc:
        with (
            tc.tile_pool(name="const", bufs=1) as constp,
            tc.tile_pool(name="io", bufs=3) as iop,
            tc.tile_pool(name="geo", bufs=3) as geop,
            tc.tile_pool(name="pfeat", bufs=2) as pfp,
            tc.tile_pool(name="trsb", bufs=3) as trsbp,
            tc.tile_pool(name="work", bufs=3) as workp,
            tc.tile_pool(name="gbig", bufs=2) as gbigp,
            tc.tile_pool(name="acc", bufs=2) as accp,
            tc.tile_pool(name="ps_tr", bufs=2, space="PSUM") as ps_tr,
            tc.tile_pool(name="ps_mm", bufs=2, space="PSUM") as ps_mm,
            tc.tile_pool(name="ps_h", bufs=1, space="PSUM") as ps_h,
            tc.tile_pool(name="ps_df", bufs=1, space="PSUM") as ps_df,
            tc.tile_pool(name="ps_g", bufs=2, space="PSUM") as ps_g,
        ):
            # ---- resident constants ----
            ident = constp.tile([P, P], F32)
            make_identity(nc, ident[:])
            ones_r = constp.tile([1, P], F32)
            nc.vector.memset(ones_r[:], 1.0)
            eps_t = constp.tile([P, 1], F32)
            nc.vector.memset(eps_t[:], 1e-5)
            nhalfpi_t = constp.tile([P, 1], F32)
            nc.vector.memset(nhalfpi_t[:], float(-np.pi / 2))
            magic_t = constp.tile([P, 1], I32)
            nc.vector.memset(magic_t[:], RSQRT_MAGIC)
            one_i = constp.tile([P, 1], I32)
            nc.vector.memset(one_i[:], 1)

            w_sb = constp.tile([P, NCHUNK, P], F32)
            nc.sync.dma_start(out=w_sb[:], in_=wflat.rearrange("(c p) w -> p c w", p=P))
            dfw1_sb = constp.tile([NB, 128], F32)
            nc.sync.dma_start(out=dfw1_sb[:], in_=dfw1)
            dfb1_sb = constp.tile([1, 128], F32)
            nc.sync.dma_start(out=dfb1_sb[:], in_=dfb1)
            dfw2gb_sb = constp.tile([128, 256], F32)
            nc.sync.dma_start(out=dfw2gb_sb[:], in_=dfw2gb)
            dfb2gb_sb = constp.tile([1, 256], F32)
            nc.sync.dma_start(out=dfb2gb_sb[:], in_=dfb2gb)
            mlpw1_sb = constp.tile([128, 512], F32)
            nc.sync.dma_start(out=mlpw1_sb[:], in_=mlpw1)
            mlpb1_sb = constp.tile([1, 512], F32)
            nc.sync.dma_start(out=mlpb1_sb[:], in_=mlpb1)
            w2rep_sb = constp.tile([P, 512], F32)
            nc.gpsimd.dma_start(out=w2rep_sb[:], in_=w2row.to_broadcast([P, 512]))
            b2_sb = constp.tile([P, 1], F32)
            nc.gpsimd.dma_start(out=b2_sb[:], in_=b2sc.to_broadcast([P, 1]))
            offs_sb = constp.tile([P, NB], F32)
            nc.gpsimd.dma_start(out=offs_sb[:], in_=offs.to_broadcast([P, NB]))

            # phase A -> phase B hand-off buffers (resident)
            rbf_store = constp.tile([P, nblocks, SUB, NB], BF16)
            dist_store = constp.tile([P, nblocks, SUB], F32)

            # =========== Phase A: geometry + RBF (exp table) ===========
            for b in range(nblocks):
                e0 = b * BLK
                sl = slice(e0, e0 + BLK)
                sidx = iop.tile([P, SUB], I32, tag="sidx")
                nc.sync.dma_start(out=sidx[:], in_=srcidx[sl].rearrange("(s p) -> p s", p=P))
                didx = iop.tile([P, SUB], I32, tag="didx")
                nc.sync.dma_start(out=didx[:], in_=dstidx[sl].rearrange("(s p) -> p s", p=P))
                esh = iop.tile([P, SUB, 3], F32, tag="esh")
                nc.sync.dma_start(out=esh[:], in_=eshift[sl, :].rearrange("(s p) j -> p s j", p=P))

                pg1 = geop.tile([P, SUB, 4], F32, tag="pg1")
                pg2 = geop.tile([P, SUB, 4], F32, tag="pg2")
                for s in range(SUB):
                    nc.gpsimd.indirect_dma_start(
                        out=pg1[:, s, :], out_offset=None, in_=posgb[:, :],
                        in_offset=IndirectOffsetOnAxis(ap=sidx[:, s : s + 1], axis=0),
                    )
                    nc.gpsimd.indirect_dma_start(
                        out=pg2[:, s, :], out_offset=None, in_=posgb[:, :],
                        in_offset=IndirectOffsetOnAxis(ap=didx[:, s : s + 1], axis=0),
                    )
                gbi = geop.tile([P, SUB], I32, tag="gbi")
                nc.vector.tensor_copy(out=gbi[:], in_=pg1[:, :, 3])
                bc = geop.tile([P, SUB, 9], F32, tag="bc")
                for s in range(SUB):
                    nc.gpsimd.indirect_dma_start(
                        out=bc[:, s, :], out_offset=None, in_=cell9[:, :],
                        in_offset=IndirectOffsetOnAxis(ap=gbi[:, s : s + 1], axis=0),
                    )

                # tvec[p,s,j] = sum_i esh[p,s,i] * bc[p,s,3i+j]
                tvp = geop.tile([P, SUB, 3, 3], F32, tag="tvp")
                nc.vector.tensor_tensor(
                    out=tvp[:],
                    in0=esh[:].unsqueeze(3).to_broadcast([P, SUB, 3, 3]),
                    in1=bc[:].rearrange("p s (i j) -> p s i j", j=3),
                    op=OP.mult,
                )
                tv = geop.tile([P, SUB, 3], F32, tag="tv")
                nc.vector.reduce_sum(
                    out=tv[:], in_=tvp[:].transpose([0, 1, 3, 2]), axis=mybir.AxisListType.X,
                )
                rv = geop.tile([P, SUB, 3], F32, tag="rv")
                nc.vector.tensor_sub(rv[:], pg2[:, :, 0:3], pg1[:, :, 0:3])
                nc.vector.tensor_add(rv[:], rv[:], tv[:])
                rv2 = geop.tile([P, SUB, 3], F32, tag="rv2")
                nc.vector.tensor_mul(rv2[:], rv[:], rv[:])
                d2 = geop.tile([P, SUB], F32, tag="d2")
                nc.vector.reduce_sum(out=d2[:], in_=rv2[:], axis=mybir.AxisListType.X)
                nc.vector.tensor_scalar(
                    out=d2[:], in0=d2[:], scalar1=1e-12, scalar2=None, op0=OP.max,
                )
                ry = _newton_rsqrt(nc, geop, d2[:], SUB, magic_t, one_i, "rsq")
                dist = dist_store[:, b, :]
                nc.vector.tensor_mul(dist, d2[:], ry[:].bitcast(F32))

                # rbf = exp(-coeff*(dist-off)^2), env applied in phase B
                rb = geop.tile([P, SUB, NB], F32, tag="rb")
                nc.vector.tensor_tensor(
                    out=rb[:],
                    in0=offs_sb[:].unsqueeze(1).to_broadcast([P, SUB, NB]),
                    in1=dist.unsqueeze(2).to_broadcast([P, SUB, NB]),
                    op=OP.subtract,
                )
                nc.scalar.activation(rb[:], rb[:], AF.Square, scale=sqc)
                nc.scalar.activation(rbf_store[:, b, :, :], rb[:], AF.Exp, scale=-1.0)

            # =========== Phase B: gathers + TP + MLPs (silu table) ===========
            for b in range(nblocks):
                e0 = b * BLK
                sl = slice(e0, e0 + BLK)
                sidx = iop.tile([P, SUB], I32, tag="sidx")
                nc.sync.dma_start(out=sidx[:], in_=srcidx[sl].rearrange("(s p) -> p s", p=P))
                didx = iop.tile([P, SUB], I32, tag="didx")
                nc.sync.dma_start(out=didx[:], in_=dstidx[sl].rearrange("(s p) -> p s", p=P))

                x1 = gbigp.tile([P, SUB, NEXT], F32, tag="x1")
                x2 = gbigp.tile([P, SUB, NEXT], F32, tag="x2")
                for s in range(SUB):
                    nc.gpsimd.indirect_dma_start(
                        out=x1[:, s, :], out_offset=None, in_=nodes_ext[:, :],
                        in_offset=IndirectOffsetOnAxis(ap=sidx[:, s : s + 1], axis=0),
                    )
                    nc.gpsimd.indirect_dma_start(
                        out=x2[:, s, :], out_offset=None, in_=nodes_ext[:, :],
                        in_offset=IndirectOffsetOnAxis(ap=didx[:, s : s + 1], axis=0),
                    )

                dist = dist_store[:, b, :]
                # envelope: cos(x) = -sin(x - pi/2), arg kept in range via min()
                dc = geop.tile([P, SUB], F32, tag="dc")
                nc.vector.tensor_scalar(
                    out=dc[:], in0=dist, scalar1=CUTOFF, scalar2=None, op0=OP.min,
                )
                cosd = geop.tile([P, SUB], F32, tag="cosd")
                nc.scalar.activation(
                    cosd[:], dc[:], AF.Sin,
                    bias=nhalfpi_t[:, 0:1], scale=float(np.pi / CUTOFF),
                )
                mask = geop.tile([P, SUB], F32, tag="mask")
                nc.vector.tensor_scalar(
                    out=mask[:], in0=dist, scalar1=CUTOFF, scalar2=None, op0=OP.is_lt,
                )
                env = geop.tile([P, SUB], F32, tag="env")
                nc.vector.tensor_scalar(
                    out=env[:], in0=cosd[:], scalar1=-0.5, scalar2=0.5,
                    op0=OP.mult, op1=OP.add,
                )
                nc.vector.tensor_mul(env[:], env[:], mask[:])
                demb = geop.tile([P, SUB, NB], F32, tag="demb")
                nc.vector.tensor_tensor(
                    out=demb[:], in0=rbf_store[:, b, :, :],
                    in1=env[:].unsqueeze(2).to_broadcast([P, SUB, NB]),
                    op=OP.mult,
                )

                mixed_sb = workp.tile([P, SUB, NS], F32, tag="mixed")
                muv = geop.tile([P, SUB], F32, tag="muv")
                varv = geop.tile([P, SUB], F32, tag="varv")

                # ---- pass 1: tensor product per sub-tile ----
                for s in range(SUB):
                    pt = pfp.tile([P, KPAD], F32, tag="pt")
                    nc.gpsimd.memset(pt[:, KTP:KPAD], 0.0)
                    a1 = x1[:, s, 0:L0]
                    a2 = x2[:, s, 0:L0]
                    nc.vector.tensor_tensor(
                        out=pt[:, 0:1024].rearrange("p (u v) -> p u v", v=L0),
                        in0=a1.unsqueeze(2).to_broadcast([P, L0, L0]),
                        in1=a2.unsqueeze(1).to_broadcast([P, L0, L0]),
                        op=OP.mult,
                    )
                    b1 = x1[:, s, 32:80].rearrange("p (u m) -> p u m", m=3)
                    b2 = x2[:, s, 32:80].rearrange("p (u m) -> p u m", m=3)
                    pb = workp.tile([P, L1, L1, 3], F32, tag="pb")
                    nc.vector.tensor_tensor(
                        out=pb[:],
                        in0=b1.unsqueeze(2).to_broadcast([P, L1, L1, 3]),
                        in1=b2.unsqueeze(1).to_broadcast([P, L1, L1, 3]),
                        op=OP.mult,
                    )
                    nc.vector.reduce_sum(
                        out=pt[:, 1024:1280].rearrange("p (u v) -> p u v", v=L1),
                        in_=pb[:], axis=mybir.AxisListType.X,
                    )
                    c1 = x1[:, s, 80:120].rearrange("p (u m) -> p u m", m=5)
                    c2 = x2[:, s, 80:120].rearrange("p (u m) -> p u m", m=5)
                    pc = workp.tile([P, L2, L2, 5], F32, tag="pc")
                    nc.vector.tensor_tensor(
                        out=pc[:],
                        in0=c1.unsqueeze(2).to_broadcast([P, L2, L2, 5]),
                        in1=c2.unsqueeze(1).to_broadcast([P, L2, L2, 5]),
                        op=OP.mult,
                    )
                    nc.vector.reduce_sum(
                        out=pt[:, 1280:1344].rearrange("p (u v) -> p u v", v=L2),
                        in_=pc[:], axis=mybir.AxisListType.X,
                    )

                    psmix = ps_mm.tile([P, NS], F32, tag="psmix")
                    for c in range(NCHUNK):
                        ptp = ps_tr.tile([P, P], F32, tag="ptp")
                        nc.tensor.transpose(ptp[:], pt[:, c * P : (c + 1) * P], ident[:])
                        pts = trsbp.tile([P, P], F32, tag="pts")
                        if c % 2 == 0:
                            nc.scalar.copy(pts[:], ptp[:])
                        else:
                            nc.vector.tensor_copy(pts[:], ptp[:])
                        nc.tensor.matmul(
                            psmix[:], lhsT=pts[:], rhs=w_sb[:, c, :],
                            start=(c == 0), stop=(c == NCHUNK - 1),
                        )

                    nc.scalar.copy(mixed_sb[:, s, :], psmix[:])
                    stats = geop.tile([P, 6], F32, tag="stats")
                    nc.vector.bn_stats(out=stats[:], in_=psmix[:])
                    mv = geop.tile([P, 2], F32, tag="mv")
                    nc.vector.bn_aggr(out=mv[:], in_=stats[:])
                    nc.vector.tensor_copy(out=muv[:, s : s + 1], in_=mv[:, 0:1])
                    nc.vector.tensor_copy(out=varv[:, s : s + 1], in_=mv[:, 1:2])

                # ---- block-level LN rstd ----
                nc.vector.tensor_scalar(
                    out=varv[:], in0=varv[:], scalar1=1e-5, scalar2=None, op0=OP.add,
                )
                ryl = _newton_rsqrt(nc, geop, varv[:], SUB, magic_t, one_i, "lnr")
                rstd = ryl[:].bitcast(F32)
                tb = geop.tile([P, SUB], F32, tag="tb")
                nc.vector.tensor_mul(tb[:], muv[:], rstd)
                nc.vector.tensor_scalar(
                    out=tb[:], in0=tb[:], scalar1=-1.0, scalar2=None, op0=OP.mult,
                )

                acc = accp.tile([P, SUB], F32, tag="acc")

                # ---- pass 2: LN apply + dfilter + final MLP ----
                for s in range(SUB):
                    ynorm = workp.tile([P, NS], F32, tag="ynorm")
                    nc.scalar.activation(
                        ynorm[:], mixed_sb[:, s, :], AF.Identity,
                        bias=tb[:, s : s + 1], scale=rstd[:, s : s + 1],
                    )

                    dT_ps = ps_tr.tile([P, P], F32, tag="ptp")
                    nc.tensor.transpose(dT_ps[0:NB, :], demb[:, s, :], ident[:])
                    dT = trsbp.tile([NB, P], F32, tag="dT")
                    nc.scalar.copy(dT[:], dT_ps[0:NB, :])
                    ph = ps_h.tile([P, 128], F32, tag="ph")
                    nc.tensor.matmul(ph[:], lhsT=ones_r[:], rhs=dfb1_sb[:], start=True, stop=False)
                    nc.tensor.matmul(ph[:], lhsT=dT[:], rhs=dfw1_sb[:], start=False, stop=True)
                    sact = workp.tile([P, 128], F32, tag="sact")
                    nc.scalar.activation(sact[:], ph[:], AF.Silu)
                    sT_ps = ps_tr.tile([P, P], F32, tag="ptp")
                    nc.tensor.transpose(sT_ps[:], sact[:], ident[:])
                    sT = trsbp.tile([P, P], F32, tag="sT")
                    nc.vector.tensor_copy(sT[:], sT_ps[:])
                    pdf = ps_df.tile([P, 256], F32, tag="pdf")
                    nc.tensor.matmul(pdf[:], lhsT=ones_r[:], rhs=dfb2gb_sb[:], start=True, stop=False)
                    nc.tensor.matmul(pdf[:], lhsT=sT[:], rhs=dfw2gb_sb[:], start=False, stop=True)

                    rg = workp.tile([P, 128], F32, tag="rg")
                    nc.vector.tensor_mul(rg[:], ynorm[:], pdf[:, 0:128])
                    nc.vector.tensor_add(rg[:], rg[:], pdf[:, 128:256])

                    rT_ps = ps_tr.tile([P, P], F32, tag="ptp")
                    nc.tensor.transpose(rT_ps[:], rg[:], ident[:])
                    rT = trsbp.tile([P, P], F32, tag="rT")
                    nc.scalar.copy(rT[:], rT_ps[:])
                    pg = ps_g.tile([P, 512], F32, tag="pg")
                    nc.tensor.matmul(pg[:], lhsT=ones_r[:], rhs=mlpb1_sb[:], start=True, stop=False)
                    nc.tensor.matmul(pg[:], lhsT=rT[:], rhs=mlpw1_sb[:], start=False, stop=True)
                    gact = gbigp.tile([P, 512], F32, tag="gact")
                    nc.scalar.activation(gact[:], pg[:], AF.Silu)
                    scr = gbigp.tile([P, 512], F32, tag="scr")
                    nc.vector.tensor_mul(scr[:], gact[:], w2rep_sb[:])
                    nc.vector.reduce_sum(
                        out=acc[:, s : s + 1], in_=scr[:], axis=mybir.AxisListType.X,
                    )

                nc.vector.tensor_scalar(
                    out=acc[:], in0=acc[:], scalar1=b2_sb[:, 0:1], scalar2=None,
                    op0=OP.add,
                )
                nc.sync.dma_start(out=out[sl].rearrange("(s p) -> p s", p=P), in_=acc[:])

    nc.compile()
    return nc


def _get_compiled():
    global _compiled
    if _compiled is None:
        _compiled = _build(NBLOCKS)
    return _compiled


def _prep(inputs):
    nodes = np.asarray(inputs["nodes"], np.float32)
    edge_index = np.asarray(inputs["edge_index"]).astype(np.int32)
    graph_batch = np.asarray(inputs["graph_batch"]).astype(np.float32)
    cell = np.asarray(inputs["cell"], np.float32)
    edge_shift = np.asarray(inputs["edge_shift"], np.float32)
    pos = np.asarray(inputs["pos"], np.float32)

    nodes_ext = np.empty((N_NODES, NEXT), np.float32)
    nodes_ext[:, :NODE_DIM] = nodes
    nodes_ext[:, 120:123] = pos
    nodes_ext[:, 123] = graph_batch

    posgb = np.empty((N_NODES, 4), np.float32)
    posgb[:, 0:3] = pos
    posgb[:, 3] = graph_batch

    alpha = 1.0 / np.sqrt(float(L0 * L0 + L1 * L1 + L2 * L2))
    w0 = np.asarray(inputs["W0"], np.float32).reshape(L0 * L0, NS) * alpha
    w1 = np.asarray(inputs["W1"], np.float32).reshape(L1 * L1, NS) * (alpha / np.sqrt(3.0))
    w2 = np.asarray(inputs["W2"], np.float32).reshape(L2 * L2, NS) * (alpha / np.sqrt(5.0))
    wflat = np.zeros((KPAD, NS), np.float32)
    wflat[0:1024] = w0
    wflat[1024:1280] = w1
    wflat[1280:1344] = w2

    ln_g = np.asarray(inputs["ln_g"], np.float32)
    ln_b = np.asarray(inputs["ln_b"], np.float32)
    df_w2 = np.asarray(inputs["df_w2"], np.float32)
    df_b2 = np.asarray(inputs["df_b2"], np.float32)
    dfw2gb = np.concatenate([df_w2 * ln_g[None, :], df_w2 * ln_b[None, :]], axis=1)
    dfb2gb = np.concatenate([df_b2 * ln_g, df_b2 * ln_b])[None, :]

    common = {
        "nodes_ext": nodes_ext,
        "posgb": posgb,
        "cell9": cell.reshape(32, 9),
        "wflat": wflat,
        "dfw1": np.asarray(inputs["df_w1"], np.float32),
        "dfb1": np.asarray(inputs["df_b1"], np.float32)[None, :],
        "dfw2gb": np.ascontiguousarray(dfw2gb),
        "dfb2gb": np.ascontiguousarray(dfb2gb),
        "mlpw1": np.asarray(inputs["mlp_w1"], np.float32),
        "mlpb1": np.asarray(inputs["mlp_b1"], np.float32)[None, :],
        "w2row": np.ascontiguousarray(np.asarray(inputs["mlp_w2"], np.float32).T),
        "b2sc": np.asarray(inputs["mlp_b2"], np.float32).reshape(1, 1),
        "offs": np.linspace(0.0, CUTOFF, NB, dtype=np.float32)[None, :],
    }

    in_maps = []
    for c in range(NCORES):
        lo, hi = c * E_CORE, (c + 1) * E_CORE
        src = np.zeros(E_PAD, np.int32)
        dst = np.zeros(E_PAD, np.int32)
        esh = np.zeros((E_PAD, 3), np.float32)
        src[: hi - lo] = edge_index[0, lo:hi]
        dst[: hi - lo] = edge_index[1, lo:hi]
        esh[: hi - lo] = edge_shift[lo:hi]
        m = dict(common)
        m["srcidx"] = src
        m["dstidx"] = dst
        m["eshift"] = esh
        in_maps.append(m)
    return in_maps


def kernel(**inputs) -> np.ndarray:
    nc = _get_compiled()
    in_maps = _prep(inputs)
    res = run_bass_kernel_spmd(nc, in_maps, core_ids=list(range(NCORES)))
    outs = [res.results[c]["out"][:E_CORE] for c in range(NCORES)]
    return np.concatenate(outs).reshape(N_EDGES, 1).astype(np.float32)
